# revision 9
# baseline (speedup 1.0000x reference)
"""Trainium2 Bass kernel for nn_Attention_7670811590880.

Multi-head attention prefill (B=1, S=2048, D=4096, H=32, KVH=8, HD=128),
tensor-parallel over heads across 8 NeuronCores.

Sharding: query head g uses kv head g % 8 (the reference's _repeat_kv inserts
the repeat axis BEFORE the kv-head axis). Core c takes query heads
{c, c+8, c+16, c+24} and kv head c; wo rows for those heads. Each core
produces a partial [S, D] output; the host sums the 8 partials.

v2 design (bf16 matmuls everywhere; fp32 PSUM accumulation):
  Phase A  per s-block (512): x chunks DMA'd once, 3 passes over the
           contraction dim (q0q1 / q2q3 / kv) with 2 interleaved PSUM
           accumulation groups each; PSUM->SBUF staging on ACT (bf16);
           RoPE on DVE (all-bf16, 2x mode); vT -> v_aug [k,129-col] via
           PE transpose, col 128 = 1.0 (softmax denominator trick).
  Phase B  per (head, q-chunk 128): causal at 128 granularity. Scores
           sT = kT_chunk.T @ qT_chunk -> PSUM f32 in groups of <=8
           chunks; one exp (ACT, scale=1/sqrt(128)) -> bf16; diagonal
           chunk masked by a precomputed 0/1 lower-tri bf16 mask (DVE
           multiply); AV with rhs = v_aug accumulates [q, 128+1] in
           PSUM: col 128 is Z. Normalize on DVE (tensor_scalar_mul by
           reciprocal of Z col), PE-transpose back to oT [hd, q].
  Phase C  out-proj: out[s_tile, n_tile] = sum_h oT_h.T @ wo_h -> DRAM.

Emission is software-pipelined: B rows of s-block j are interleaved with
phase-A passes of s-block j+1 (and B rows of the last block with phase-C
tiles) so ACT exp time hides under PE-bound GEMM work.
"""
import math
from contextlib import ExitStack

import numpy as np
import ml_dtypes

import concourse.bass as bass
import concourse.tile as tile
from concourse import bacc, mybir
from concourse.bass import ds, ts
from concourse.bass_utils import run_bass_kernel_spmd
from concourse.masks import make_identity

P = 128          # partitions / head_dim
SB = 512         # s-block width
F32 = mybir.dt.float32
BF16 = mybir.dt.bfloat16

# problem constants
B, S, D = 1, 2048, 4096
H, KVH, HD = 32, 8, 128
NCORES = 8
NQH = H // NCORES      # q heads per core = 4
ROPE_HALF = HD // 2    # 64


def _interleave(ga, gb, b_per_a):
    """Pull from generator ga once, then ~b_per_a times from gb, until both
    are exhausted. Emission order only; correctness comes from tile deps."""
    err = 0.0
    a_live, b_live = True, True
    while a_live or b_live:
        if a_live:
            try:
                next(ga)
            except StopIteration:
                a_live = False
        err += b_per_a
        n = int(err)
        err -= n
        if not a_live:
            n = 1 << 30
        for _ in range(n):
            if not b_live:
                break
            try:
                next(gb)
            except StopIteration:
                b_live = False
                break


def build_attention_kernel(S_=S, D_=D, nqh=NQH, loop_reps=None,
                           phases="ABC"):
    """Build the per-core Bass kernel. Returns compiled Bacc object.

    Inputs (per core, DRAM):
      xT      [D_, S_]        bf16  x transposed
      wq      [D_, nqh*128]   bf16  q weights, head-grouped, rope-permuted cols
      wk      [D_, 128]       bf16  rope-permuted cols
      wv      [D_, 128]       bf16
      wo      [nqh*128, D_]   bf16
      cosT    [128, S_]       bf16  cos||cos
      sinT    [128, S_]       bf16  -sin||sin
    Output:
      out     [S_, D_]        f32   partial (this core's heads through wo)
    """
    DC = D_ // P           # d chunks = 32
    SBLK = S_ // SB        # s blocks = 4
    CPB = SB // P          # 128-chunks per block = 4
    SSUB = S_ // P         # s subtiles = 16
    NT = D_ // SB          # out-proj n tiles = 8
    GRP = 4                # k-chunks per score/exp group
    inv_sqrt_hd = 1.0 / math.sqrt(HD)

    nc = bacc.Bacc("TRN2", target_bir_lowering=False, debug=False,
                   num_devices=NCORES)
    xT = nc.dram_tensor("xT", [D_, S_], BF16, kind="ExternalInput").ap()
    wq = nc.dram_tensor("wq", [D_, nqh * P], BF16, kind="ExternalInput").ap()
    wk = nc.dram_tensor("wk", [D_, P], BF16, kind="ExternalInput").ap()
    wv = nc.dram_tensor("wv", [D_, P], BF16, kind="ExternalInput").ap()
    wo = nc.dram_tensor("wo", [nqh * P, D_], BF16, kind="ExternalInput").ap()
    cosT = nc.dram_tensor("cosT", [P, S_], BF16, kind="ExternalInput").ap()
    sinT = nc.dram_tensor("sinT", [P, S_], BF16, kind="ExternalInput").ap()
    out = nc.dram_tensor("out", [S_, D_], F32, kind="ExternalOutput").ap()

    with tile.TileContext(nc) as tc, ExitStack() as top:
        persist = top.enter_context(tc.tile_pool(name="persist", bufs=1))

        def body():
            with ExitStack() as ctx:
                # ---- per-block persistent SBUF (distinct tags => precise
                # block-level dependencies for the software pipeline) ----
                qT = [persist.tile([P, nqh, SB], BF16, tag=f"qT{j}",
                                   name=f"qT{j}") for j in range(SBLK)]
                kT = [persist.tile([P, SB], BF16, tag=f"kT{j}",
                                   name=f"kT{j}") for j in range(SBLK)]
                va = [persist.tile([P, CPB, P + 4], BF16, tag=f"va{j}",
                                   name=f"va{j}") for j in range(SBLK)]
                oT = [persist.tile([P, nqh, SB], BF16, tag=f"oT{j}",
                                   name=f"oT{j}") for j in range(SBLK)]
                ident_f = persist.tile([P, P], F32, tag="idf", name="ident_f")
                ident_b = persist.tile([P, P], BF16, tag="idb", name="ident_b")
                mask_f = persist.tile([P, P], F32, tag="mkf", name="mask_f")
                mask_b = persist.tile([P, P], BF16, tag="mkb", name="mask_b")

                make_identity(nc, ident_f[:])
                nc.vector.tensor_copy(ident_b[:], ident_f[:])
                # lower-triangular-inclusive 0/1 mask: keep q >= k
                # (partition = k, free = q)
                nc.gpsimd.memset(mask_f[:], 1.0)
                nc.gpsimd.affine_select(
                    mask_f[:], mask_f[:], pattern=[[1, P]],
                    compare_op=mybir.AluOpType.is_ge, fill=0.0,
                    base=0, channel_multiplier=-1)
                nc.vector.tensor_copy(mask_b[:], mask_f[:])
                for j in range(SBLK):
                    nc.vector.memset(va[j][:], 1.0)

                attn = ctx.enter_context(ExitStack())
                psT = attn.enter_context(
                    tc.tile_pool(name="psT", bufs=2, space="PSUM"))
                psS = attn.enter_context(
                    tc.tile_pool(name="psS", bufs=2, space="PSUM"))
                psO = attn.enter_context(
                    tc.tile_pool(name="psO", bufs=2, space="PSUM"))
                epool = attn.enter_context(tc.tile_pool(name="epool", bufs=2))
                zpool = attn.enter_context(tc.tile_pool(name="zpool", bufs=2))
                onpool = attn.enter_context(tc.tile_pool(name="onp", bufs=2))

                # ================= phase A =================
                def a_block(j, apool, xpool, stgp, rpool, wq_sb, wk_sb, wv_sb):
                    xb = xpool.tile([P, DC, SB], BF16, tag="xb", name="xb")
                    for dc in range(DC):
                        nc.sync.dma_start(
                            xb[:, dc, :], xT[ds(dc * P, P), ds(j * SB, SB)])
                    cc = rpool.tile([P, SB], BF16, tag="cc", name="cc")
                    ss = rpool.tile([P, SB], BF16, tag="ss", name="ss")
                    nc.sync.dma_start(cc[:], cosT[:, ds(j * SB, SB)])
                    nc.sync.dma_start(ss[:], sinT[:, ds(j * SB, SB)])

                    def rope(dst, src):
                        rot = rpool.tile([P, SB], BF16, tag="rot", name="rot")
                        tm = rpool.tile([P, SB], BF16, tag="tm", name="tm")
                        nc.vector.tensor_copy(rot[0:ROPE_HALF, :],
                                              src[ROPE_HALF:P, :])
                        nc.vector.tensor_copy(rot[ROPE_HALF:P, :],
                                              src[0:ROPE_HALF, :])
                        nc.vector.tensor_mul(tm[:], rot[:], ss[:])
                        nc.vector.tensor_mul(dst, src, cc[:])
                        nc.vector.tensor_tensor(dst, dst, tm[:],
                                                mybir.AluOpType.add)

                    # pass 1: q0,q1  pass 2: q2,q3  pass 3: k,v
                    for pss in range(3):
                        pa = apool.tile([P, SB], F32, tag="psA", name="pa")
                        pb = apool.tile([P, SB], F32, tag="psA", name="pb")
                        for dc in range(DC):
                            if pss < 2:
                                nc.tensor.matmul(
                                    pa[:], wq_sb[:, dc, ts(2 * pss, P)],
                                    xb[:, dc, :],
                                    start=(dc == 0), stop=(dc == DC - 1))
                                nc.tensor.matmul(
                                    pb[:], wq_sb[:, dc, ts(2 * pss + 1, P)],
                                    xb[:, dc, :],
                                    start=(dc == 0), stop=(dc == DC - 1))
                            else:
                                nc.tensor.matmul(
                                    pa[:], wk_sb[:, dc, :], xb[:, dc, :],
                                    start=(dc == 0), stop=(dc == DC - 1))
                                nc.tensor.matmul(
                                    pb[:], wv_sb[:, dc, :], xb[:, dc, :],
                                    start=(dc == 0), stop=(dc == DC - 1))
                            if dc % 4 == 3:
                                yield
                        stg = stgp.tile([P, 2, SB], BF16, tag="stg",
                                        name="stg")
                        nc.scalar.copy(stg[:, 0, :], pa[:])
                        nc.scalar.copy(stg[:, 1, :], pb[:])
                        if pss < 2:
                            rope(qT[j][:, 2 * pss, :], stg[:, 0, :])
                            rope(qT[j][:, 2 * pss + 1, :], stg[:, 1, :])
                        else:
                            rope(kT[j][:, :], stg[:, 0, :])
                            for st in range(CPB):
                                pt = psT.tile([P, P], BF16, tag="psT",
                                              name="pt")
                                nc.tensor.transpose(
                                    pt[:], stg[:, 1, ts(st, P)], ident_b[:])
                                nc.vector.tensor_copy(va[j][:, st, 0:P], pt[:])
                        yield

                # ================= phase B =================
                # Software-pipelined: scores/exp of group g+1 are emitted
                # before the AV matmuls of group g, so the PE never sits
                # behind an ACT-exp wait; the oT transpose of a (h,qc) row
                # is deferred into the next row for the same reason.
                def b_rows(j):
                    pend_tr = []    # [(onrm, h, qcl)]

                    def flush_tr():
                        while pend_tr:
                            onrm, h_, q_ = pend_tr.pop(0)
                            pt = psT.tile([P, P], BF16, tag="psT", name="ptB")
                            nc.tensor.transpose(pt[:], onrm[:], ident_b[:])
                            nc.vector.tensor_copy(oT[j][:, h_, ts(q_, P)],
                                                  pt[:])

                    def emit_avs(po, e2, kc0, glen, nk):
                        for u in range(glen):
                            kc = kc0 + u
                            nc.tensor.matmul(
                                po[:, 0:P + 1], e2[:, u, :],
                                va[kc // CPB][:, kc % CPB, 0:P + 1],
                                start=(kc == 0), stop=(kc == nk - 1))

                    for h in range(nqh):
                        for qcl in range(CPB):
                            qc = CPB * j + qcl
                            nk = qc + 1
                            po = psO.tile([P, P + 1], F32, tag="po", name="po")
                            pend_av = None
                            for kc0 in range(0, nk, GRP):
                                glen = min(GRP, nk - kc0)
                                ps_s = psS.tile([P, GRP, P], F32, tag="pss",
                                                name="ps_s")
                                for u in range(glen):
                                    kc = kc0 + u
                                    nc.tensor.matmul(
                                        ps_s[:, u, :],
                                        kT[kc // CPB][:, ts(kc % CPB, P)],
                                        qT[j][:, h, ts(qcl, P)],
                                        start=True, stop=True)
                                e2 = epool.tile([P, GRP, P], BF16, tag="e2",
                                                name="e2")
                                nc.scalar.activation(
                                    e2[:, 0:glen, :], ps_s[:, 0:glen, :],
                                    mybir.ActivationFunctionType.Exp,
                                    scale=inv_sqrt_hd)
                                if kc0 + glen == nk:
                                    # last chunk is the diagonal: causal mask
                                    nc.vector.tensor_mul(
                                        e2[:, glen - 1, :], e2[:, glen - 1, :],
                                        mask_b[:])
                                if kc0 == 0:
                                    flush_tr()
                                yield
                                if pend_av is not None:
                                    emit_avs(po, *pend_av, nk)
                                    yield
                                pend_av = (e2, kc0, glen)
                            emit_avs(po, *pend_av, nk)
                            zr = zpool.tile([P, 1], F32, tag="zr", name="zr")
                            nc.vector.reciprocal(zr[:], po[:, P:P + 1])
                            onrm = onpool.tile([P, P], BF16, tag="on",
                                               name="onrm")
                            nc.vector.tensor_scalar_mul(onrm[:], po[:, 0:P],
                                                        zr[:])
                            pend_tr.append((onrm, h, qcl))
                            yield
                    flush_tr()

                # ================= phase C =================
                def c_tiles(st_list, psC, copool, wo_sb):
                    for st in st_list:
                        jj, stl = st // CPB, st % CPB
                        for nt in range(NT):
                            pc = psC.tile([P, SB], F32, tag="pc", name="pc")
                            for hh in range(nqh):
                                nc.tensor.matmul(
                                    pc[:], oT[jj][:, hh, ts(stl, P)],
                                    wo_sb[:, hh, ts(nt, SB)],
                                    start=(hh == 0), stop=(hh == nqh - 1))
                            ot = copool.tile([P, SB], F32, tag="ot", name="ot")
                            nc.any.tensor_copy(ot[:], pc[:])
                            nc.sync.dma_start(
                                out[ds(st * P, P), ds(nt * SB, SB)], ot[:])
                            yield

                # ---- segment 1: A blocks pipelined with B rows ----
                with ExitStack() as actx:
                    wpool = actx.enter_context(
                        tc.tile_pool(name="wpool", bufs=1))
                    xpool = actx.enter_context(
                        tc.tile_pool(name="xpool", bufs=2))
                    stgp = actx.enter_context(tc.tile_pool(name="stgp",
                                                           bufs=2))
                    rpool = actx.enter_context(tc.tile_pool(name="rpool",
                                                            bufs=2))
                    apool = actx.enter_context(
                        tc.tile_pool(name="apool", bufs=2, space="PSUM"))

                    wq_sb = wpool.tile([P, DC, nqh * P], BF16, tag="wq",
                                       name="wq_sb")
                    wk_sb = wpool.tile([P, DC, P], BF16, tag="wk",
                                       name="wk_sb")
                    wv_sb = wpool.tile([P, DC, P], BF16, tag="wv",
                                       name="wv_sb")
                    # per-chunk weight DMAs so the first matmuls only wait
                    # on their own chunk, not the full weight transfer
                    for dc in range(DC):
                        nc.sync.dma_start(wq_sb[:, dc, :],
                                          wq[ds(dc * P, P), :])
                        nc.sync.dma_start(wk_sb[:, dc, :],
                                          wk[ds(dc * P, P), :])
                        nc.sync.dma_start(wv_sb[:, dc, :],
                                          wv[ds(dc * P, P), :])

                    def ab(jj):
                        return a_block(jj, apool, xpool, stgp, rpool,
                                       wq_sb, wk_sb, wv_sb)

                    for _ in ab(0):
                        pass
                    _interleave(ab(1), b_rows(0), 1.2)
                    _interleave(ab(2), b_rows(1), 2.4)
                    _interleave(ab(3), b_rows(2), 3.6)

                # ---- segment 2: C tiles pipelined with last B rows ----
                with ExitStack() as cctx:
                    wopool = cctx.enter_context(
                        tc.tile_pool(name="wopool", bufs=1))
                    copool = cctx.enter_context(
                        tc.tile_pool(name="copool", bufs=4))
                    psC = cctx.enter_context(
                        tc.tile_pool(name="psC", bufs=2, space="PSUM"))
                    wo_sb = wopool.tile([P, nqh, D_], BF16, tag="wo",
                                        name="wo_sb")
                    nc.sync.dma_start(wo_sb[:],
                                      wo.rearrange("(o p) m -> p o m", p=P))
                    _interleave(b_rows(3),
                                c_tiles(range(0, 12), psC, copool, wo_sb),
                                0.75)
                    for _ in c_tiles(range(12, SSUB), psC, copool, wo_sb):
                        pass

        if loop_reps is not None:
            with tc.For_i(0, loop_reps, 1):
                body()
        else:
            body()

    nc.compile()
    return nc


_ROPE_PERM = np.concatenate([np.arange(0, HD, 2), np.arange(1, HD, 2)])


def shard_inputs(x, wq, wk, wv, wo, freqs_cos, freqs_sin):
    """Host-side sharding/layout. Returns list of 8 per-core input dicts."""
    bf = ml_dtypes.bfloat16
    x2 = np.asarray(x, dtype=np.float32).reshape(S, D)
    xTh = np.ascontiguousarray(x2.T.astype(bf))                   # [D, S]
    cos_h = np.asarray(freqs_cos, np.float32).T                   # [64, S]
    sin_h = np.asarray(freqs_sin, np.float32).T
    cosT = np.ascontiguousarray(
        np.concatenate([cos_h, cos_h], axis=0).astype(bf))
    sinT = np.ascontiguousarray(
        np.concatenate([-sin_h, sin_h], axis=0).astype(bf))
    wq = np.asarray(wq, np.float32)
    wk = np.asarray(wk, np.float32)
    wv = np.asarray(wv, np.float32)
    wo = np.asarray(wo, np.float32)
    in_maps = []
    for c in range(NCORES):
        heads = [c + NCORES * r for r in range(NQH)]       # g % KVH == c
        wq_c = np.concatenate(
            [wq[:, g * HD + _ROPE_PERM] for g in heads], axis=1)
        wk_c = wk[:, c * HD + _ROPE_PERM]
        wv_c = wv[:, c * HD:(c + 1) * HD]
        wo_c = np.concatenate([wo[g * HD:(g + 1) * HD, :] for g in heads],
                              axis=0)
        in_maps.append({
            "xT": xTh,
            "wq": np.ascontiguousarray(wq_c.astype(bf)),
            "wk": np.ascontiguousarray(wk_c.astype(bf)),
            "wv": np.ascontiguousarray(wv_c.astype(bf)),
            "wo": np.ascontiguousarray(wo_c.astype(bf)),
            "cosT": cosT,
            "sinT": sinT,
        })
    return in_maps


_NC_CACHE = {}


def _get_nc():
    if "nc" not in _NC_CACHE:
        _NC_CACHE["nc"] = build_attention_kernel()
    return _NC_CACHE["nc"]


def kernel(x, wq, wk, wv, wo, freqs_cos, freqs_sin, mask, cache_k, cache_v,
           start_pos):
    assert int(start_pos) == 0, "kernel assumes prefill at start_pos=0"
    in_maps = shard_inputs(x, wq, wk, wv, wo, freqs_cos, freqs_sin)
    nc = _get_nc()
    res = run_bass_kernel_spmd(nc, in_maps, core_ids=list(range(NCORES)))
    acc = np.zeros((S, D), np.float32)
    for c in range(NCORES):
        acc += res.results[c]["out"]
    return acc.reshape(B, S, D)


# revision 11
# speedup vs baseline: 1.3298x; 1.3298x over previous
"""Trainium2 Bass kernel for nn_Attention_7670811590880.

Multi-head attention prefill (B=1, S=2048, D=4096, H=32, KVH=8, HD=128),
tensor-parallel over heads across 8 NeuronCores.

Sharding: query head g uses kv head g % 8 (the reference's _repeat_kv inserts
the repeat axis BEFORE the kv-head axis). Core c takes query heads
{c, c+8, c+16, c+24} and kv head c; wo rows for those heads. Each core
produces a partial [S, D] output; the host sums the 8 partials.

v2 design (bf16 matmuls everywhere; fp32 PSUM accumulation):
  Phase A  per s-block (512): x chunks DMA'd once, 3 passes over the
           contraction dim (q0q1 / q2q3 / kv) with 2 interleaved PSUM
           accumulation groups each; PSUM->SBUF staging on ACT (bf16);
           RoPE on DVE (all-bf16, 2x mode); vT -> v_aug [k,129-col] via
           PE transpose, col 128 = 1.0 (softmax denominator trick).
  Phase B  per (head, q-chunk 128): causal at 128 granularity. Scores
           sT = kT_chunk.T @ qT_chunk -> PSUM f32 in groups of <=8
           chunks; one exp (ACT, scale=1/sqrt(128)) -> bf16; diagonal
           chunk masked by a precomputed 0/1 lower-tri bf16 mask (DVE
           multiply); AV with rhs = v_aug accumulates [q, 128+1] in
           PSUM: col 128 is Z. Normalize on DVE (tensor_scalar_mul by
           reciprocal of Z col), PE-transpose back to oT [hd, q].
  Phase C  out-proj: out[s_tile, n_tile] = sum_h oT_h.T @ wo_h -> DRAM.

Emission is software-pipelined: B rows of s-block j are interleaved with
phase-A passes of s-block j+1 (and B rows of the last block with phase-C
tiles) so ACT exp time hides under PE-bound GEMM work.
"""
import math
from contextlib import ExitStack

import numpy as np
import ml_dtypes

import concourse.bass as bass
import concourse.tile as tile
from concourse import bacc, mybir
from concourse.bass import ds, ts
from concourse.bass_utils import run_bass_kernel_spmd
from concourse.masks import make_identity

P = 128          # partitions / head_dim
SB = 512         # s-block width
F32 = mybir.dt.float32
BF16 = mybir.dt.bfloat16

# problem constants
B, S, D = 1, 2048, 4096
H, KVH, HD = 32, 8, 128
NCORES = 8
NQH = H // NCORES      # q heads per core = 4
ROPE_HALF = HD // 2    # 64


def _interleave(ga, gb, b_per_a):
    """Pull from generator ga once, then ~b_per_a times from gb, until both
    are exhausted. Emission order only; correctness comes from tile deps."""
    err = 0.0
    a_live, b_live = True, True
    while a_live or b_live:
        if a_live:
            try:
                next(ga)
            except StopIteration:
                a_live = False
        err += b_per_a
        n = int(err)
        err -= n
        if not a_live:
            n = 1 << 30
        for _ in range(n):
            if not b_live:
                break
            try:
                next(gb)
            except StopIteration:
                b_live = False
                break


def build_attention_kernel(S_=S, D_=D, nqh=NQH, loop_reps=None,
                           phases="ABC"):
    """Build the per-core Bass kernel. Returns compiled Bacc object.

    Inputs (per core, DRAM):
      xT      [D_, S_]        bf16  x transposed
      wq      [D_, nqh*128]   bf16  q weights, head-grouped, rope-permuted cols
      wk      [D_, 128]       bf16  rope-permuted cols
      wv      [D_, 128]       bf16
      wo      [nqh*128, D_]   bf16
      cosT    [128, S_]       bf16  cos||cos
      sinT    [128, S_]       bf16  -sin||sin
    Output:
      out     [S_, D_]        f32   partial (this core's heads through wo)
    """
    DC = D_ // P           # d chunks = 32
    SBLK = S_ // SB        # s blocks = 4
    CPB = SB // P          # 128-chunks per block = 4
    SSUB = S_ // P         # s subtiles = 16
    NT = D_ // SB          # out-proj n tiles = 8
    GRP = 4                # k-chunks per score/exp group
    inv_sqrt_hd = 1.0 / math.sqrt(HD)

    nc = bacc.Bacc("TRN2", target_bir_lowering=False, debug=False,
                   num_devices=NCORES)
    xT = nc.dram_tensor("xT", [D_, S_], BF16, kind="ExternalInput").ap()
    wq = nc.dram_tensor("wq", [D_, nqh * P], BF16, kind="ExternalInput").ap()
    wk = nc.dram_tensor("wk", [D_, P], BF16, kind="ExternalInput").ap()
    wv = nc.dram_tensor("wv", [D_, P], BF16, kind="ExternalInput").ap()
    wo = nc.dram_tensor("wo", [nqh * P, D_], BF16, kind="ExternalInput").ap()
    cosT = nc.dram_tensor("cosT", [P, S_], BF16, kind="ExternalInput").ap()
    sinT = nc.dram_tensor("sinT", [P, S_], BF16, kind="ExternalInput").ap()
    out = nc.dram_tensor("out", [S_, D_], F32, kind="ExternalOutput").ap()

    with tile.TileContext(nc) as tc, ExitStack() as top:
        persist = top.enter_context(tc.tile_pool(name="persist", bufs=1))

        def body():
            with ExitStack() as ctx:
                # ---- per-block persistent SBUF (distinct tags => precise
                # block-level dependencies for the software pipeline) ----
                qT = [persist.tile([P, nqh, SB], BF16, tag=f"qT{j}",
                                   name=f"qT{j}") for j in range(SBLK)]
                kT = [persist.tile([P, SB], BF16, tag=f"kT{j}",
                                   name=f"kT{j}") for j in range(SBLK)]
                va = [persist.tile([P, CPB, P + 4], BF16, tag=f"va{j}",
                                   name=f"va{j}") for j in range(SBLK)]
                oT = [persist.tile([P, nqh, SB], BF16, tag=f"oT{j}",
                                   name=f"oT{j}") for j in range(SBLK)]
                ident_f = persist.tile([P, P], F32, tag="idf", name="ident_f")
                ident_b = persist.tile([P, P], BF16, tag="idb", name="ident_b")
                mask_f = persist.tile([P, P], F32, tag="mkf", name="mask_f")
                mask_b = persist.tile([P, P], BF16, tag="mkb", name="mask_b")

                make_identity(nc, ident_f[:])
                nc.vector.tensor_copy(ident_b[:], ident_f[:])
                # lower-triangular-inclusive 0/1 mask: keep q >= k
                # (partition = k, free = q)
                nc.gpsimd.memset(mask_f[:], 1.0)
                nc.gpsimd.affine_select(
                    mask_f[:], mask_f[:], pattern=[[1, P]],
                    compare_op=mybir.AluOpType.is_ge, fill=0.0,
                    base=0, channel_multiplier=-1)
                nc.vector.tensor_copy(mask_b[:], mask_f[:])
                for j in range(SBLK):
                    nc.vector.memset(va[j][:], 1.0)

                attn = ctx.enter_context(ExitStack())
                psT = attn.enter_context(
                    tc.tile_pool(name="psT", bufs=2, space="PSUM"))
                psS = attn.enter_context(
                    tc.tile_pool(name="psS", bufs=2, space="PSUM"))
                psO = attn.enter_context(
                    tc.tile_pool(name="psO", bufs=2, space="PSUM"))
                epool = attn.enter_context(tc.tile_pool(name="epool", bufs=2))
                zpool = attn.enter_context(tc.tile_pool(name="zpool", bufs=2))
                onpool = attn.enter_context(tc.tile_pool(name="onp", bufs=2))

                # ================= phase A =================
                def a_block(j, apool, xpool, stgp, rpool, wq_sb, wk_sb, wv_sb):
                    xb = xpool.tile([P, DC, SB], BF16, tag="xb", name="xb")
                    for dc in range(DC):
                        nc.sync.dma_start(
                            xb[:, dc, :], xT[ds(dc * P, P), ds(j * SB, SB)])
                    cc = rpool.tile([P, SB], BF16, tag="cc", name="cc")
                    ss = rpool.tile([P, SB], BF16, tag="ss", name="ss")
                    nc.sync.dma_start(cc[:], cosT[:, ds(j * SB, SB)])
                    nc.sync.dma_start(ss[:], sinT[:, ds(j * SB, SB)])

                    def rope(dst, src):
                        rot = rpool.tile([P, SB], BF16, tag="rot", name="rot")
                        tm = rpool.tile([P, SB], BF16, tag="tm", name="tm")
                        nc.vector.tensor_copy(rot[0:ROPE_HALF, :],
                                              src[ROPE_HALF:P, :])
                        nc.vector.tensor_copy(rot[ROPE_HALF:P, :],
                                              src[0:ROPE_HALF, :])
                        nc.vector.tensor_mul(tm[:], rot[:], ss[:])
                        nc.vector.tensor_mul(dst, src, cc[:])
                        nc.vector.tensor_tensor(dst, dst, tm[:],
                                                mybir.AluOpType.add)

                    # pass 1: q0,q1  pass 2: q2,q3  pass 3: k,v
                    for pss in range(3):
                        pa = apool.tile([P, SB], F32, tag="psA", name="pa")
                        pb = apool.tile([P, SB], F32, tag="psA", name="pb")
                        for dc in range(DC):
                            if pss < 2:
                                nc.tensor.matmul(
                                    pa[:], wq_sb[:, dc, ts(2 * pss, P)],
                                    xb[:, dc, :],
                                    start=(dc == 0), stop=(dc == DC - 1))
                                nc.tensor.matmul(
                                    pb[:], wq_sb[:, dc, ts(2 * pss + 1, P)],
                                    xb[:, dc, :],
                                    start=(dc == 0), stop=(dc == DC - 1))
                            else:
                                nc.tensor.matmul(
                                    pa[:], wk_sb[:, dc, :], xb[:, dc, :],
                                    start=(dc == 0), stop=(dc == DC - 1))
                                nc.tensor.matmul(
                                    pb[:], wv_sb[:, dc, :], xb[:, dc, :],
                                    start=(dc == 0), stop=(dc == DC - 1))
                            if dc % 4 == 3:
                                yield
                        stg = stgp.tile([P, 2, SB], BF16, tag="stg",
                                        name="stg")
                        nc.scalar.copy(stg[:, 0, :], pa[:])
                        nc.scalar.copy(stg[:, 1, :], pb[:])
                        if pss < 2:
                            rope(qT[j][:, 2 * pss, :], stg[:, 0, :])
                            rope(qT[j][:, 2 * pss + 1, :], stg[:, 1, :])
                        else:
                            rope(kT[j][:, :], stg[:, 0, :])
                            for st in range(CPB):
                                pt = psT.tile([P, P], BF16, tag="psT",
                                              name="pt")
                                nc.tensor.transpose(
                                    pt[:], stg[:, 1, ts(st, P)], ident_b[:])
                                nc.vector.tensor_copy(va[j][:, st, 0:P], pt[:])
                        yield

                # ================= phase B =================
                # Software-pipelined: scores/exp of group g+1 are emitted
                # before the AV matmuls of group g, so the PE never sits
                # behind an ACT-exp wait; the oT transpose of a (h,qc) row
                # is deferred into the next row for the same reason.
                def b_rows(j):
                    pend_tr = []    # [(onrm, h, qcl)]

                    def flush_tr():
                        while pend_tr:
                            onrm, h_, q_ = pend_tr.pop(0)
                            pt = psT.tile([P, P], BF16, tag="psT", name="ptB")
                            nc.tensor.transpose(pt[:], onrm[:], ident_b[:])
                            nc.vector.tensor_copy(oT[j][:, h_, ts(q_, P)],
                                                  pt[:])

                    def emit_avs(po, e2, kc0, glen, nk):
                        for u in range(glen):
                            kc = kc0 + u
                            nc.tensor.matmul(
                                po[:, 0:P + 1], e2[:, u, :],
                                va[kc // CPB][:, kc % CPB, 0:P + 1],
                                start=(kc == 0), stop=(kc == nk - 1))

                    for h in range(nqh):
                        for qcl in range(CPB):
                            qc = CPB * j + qcl
                            nk = qc + 1
                            po = psO.tile([P, P + 1], F32, tag="po", name="po")
                            pend_av = None
                            for kc0 in range(0, nk, GRP):
                                glen = min(GRP, nk - kc0)
                                ps_s = psS.tile([P, GRP, P], F32, tag="pss",
                                                name="ps_s")
                                for u in range(glen):
                                    kc = kc0 + u
                                    nc.tensor.matmul(
                                        ps_s[:, u, :],
                                        kT[kc // CPB][:, ts(kc % CPB, P)],
                                        qT[j][:, h, ts(qcl, P)],
                                        start=True, stop=True)
                                e2 = epool.tile([P, GRP, P], BF16, tag="e2",
                                                name="e2")
                                nc.scalar.activation(
                                    e2[:, 0:glen, :], ps_s[:, 0:glen, :],
                                    mybir.ActivationFunctionType.Exp,
                                    scale=inv_sqrt_hd)
                                if kc0 + glen == nk:
                                    # last chunk is the diagonal: causal mask
                                    nc.vector.tensor_mul(
                                        e2[:, glen - 1, :], e2[:, glen - 1, :],
                                        mask_b[:])
                                if kc0 == 0:
                                    flush_tr()
                                yield
                                if pend_av is not None:
                                    emit_avs(po, *pend_av, nk)
                                    yield
                                pend_av = (e2, kc0, glen)
                            emit_avs(po, *pend_av, nk)
                            zr = zpool.tile([P, 1], F32, tag="zr", name="zr")
                            nc.vector.reciprocal(zr[:], po[:, P:P + 1])
                            onrm = onpool.tile([P, P], BF16, tag="on",
                                               name="onrm")
                            nc.vector.tensor_scalar_mul(onrm[:], po[:, 0:P],
                                                        zr[:])
                            pend_tr.append((onrm, h, qcl))
                            yield
                    flush_tr()

                # ================= phase C =================
                def c_tiles(st_list, psC, copool, wo_sb):
                    for st in st_list:
                        jj, stl = st // CPB, st % CPB
                        for nt in range(NT):
                            pc = psC.tile([P, SB], F32, tag="pc", name="pc")
                            for hh in range(nqh):
                                nc.tensor.matmul(
                                    pc[:], oT[jj][:, hh, ts(stl, P)],
                                    wo_sb[:, hh, ts(nt, SB)],
                                    start=(hh == 0), stop=(hh == nqh - 1))
                            ot = copool.tile([P, SB], F32, tag="ot", name="ot")
                            if nt % 2 == 0:
                                nc.scalar.copy(ot[:], pc[:])
                            else:
                                nc.vector.tensor_copy(ot[:], pc[:])
                            nc.sync.dma_start(
                                out[ds(st * P, P), ds(nt * SB, SB)], ot[:])
                            yield

                # ---- segment 1: A blocks pipelined with B rows ----
                with ExitStack() as actx:
                    wpool = actx.enter_context(
                        tc.tile_pool(name="wpool", bufs=1))
                    xpool = actx.enter_context(
                        tc.tile_pool(name="xpool", bufs=2))
                    stgp = actx.enter_context(tc.tile_pool(name="stgp",
                                                           bufs=2))
                    rpool = actx.enter_context(tc.tile_pool(name="rpool",
                                                            bufs=2))
                    apool = actx.enter_context(
                        tc.tile_pool(name="apool", bufs=2, space="PSUM"))

                    wq_sb = wpool.tile([P, DC, nqh * P], BF16, tag="wq",
                                       name="wq_sb")
                    wk_sb = wpool.tile([P, DC, P], BF16, tag="wk",
                                       name="wk_sb")
                    wv_sb = wpool.tile([P, DC, P], BF16, tag="wv",
                                       name="wv_sb")
                    # per-chunk weight DMAs so the first matmuls only wait
                    # on their own chunk, not the full weight transfer
                    for dc in range(DC):
                        nc.sync.dma_start(wq_sb[:, dc, :],
                                          wq[ds(dc * P, P), :])
                        nc.sync.dma_start(wk_sb[:, dc, :],
                                          wk[ds(dc * P, P), :])
                        nc.sync.dma_start(wv_sb[:, dc, :],
                                          wv[ds(dc * P, P), :])

                    def ab(jj):
                        return a_block(jj, apool, xpool, stgp, rpool,
                                       wq_sb, wk_sb, wv_sb)

                    for _ in ab(0):
                        pass
                    _interleave(ab(1), b_rows(0), 1.2)
                    _interleave(ab(2), b_rows(1), 2.4)
                    _interleave(ab(3), b_rows(2), 3.6)

                # ---- segment 2: C tiles pipelined with last B rows ----
                with ExitStack() as cctx:
                    wopool = cctx.enter_context(
                        tc.tile_pool(name="wopool", bufs=1))
                    copool = cctx.enter_context(
                        tc.tile_pool(name="copool", bufs=4))
                    psC = cctx.enter_context(
                        tc.tile_pool(name="psC", bufs=2, space="PSUM"))
                    wo_sb = wopool.tile([P, nqh, D_], BF16, tag="wo",
                                        name="wo_sb")
                    nc.sync.dma_start(wo_sb[:],
                                      wo.rearrange("(o p) m -> p o m", p=P))
                    _interleave(b_rows(3),
                                c_tiles(range(0, 12), psC, copool, wo_sb),
                                0.75)
                    for _ in c_tiles(range(12, SSUB), psC, copool, wo_sb):
                        pass

        if loop_reps is not None:
            with tc.For_i(0, loop_reps, 1):
                body()
        else:
            body()

    nc.compile()
    return nc


_ROPE_PERM = np.concatenate([np.arange(0, HD, 2), np.arange(1, HD, 2)])


def shard_inputs(x, wq, wk, wv, wo, freqs_cos, freqs_sin):
    """Host-side sharding/layout. Returns list of 8 per-core input dicts."""
    bf = ml_dtypes.bfloat16
    x2 = np.asarray(x, dtype=np.float32).reshape(S, D)
    xTh = np.ascontiguousarray(x2.T.astype(bf))                   # [D, S]
    cos_h = np.asarray(freqs_cos, np.float32).T                   # [64, S]
    sin_h = np.asarray(freqs_sin, np.float32).T
    cosT = np.ascontiguousarray(
        np.concatenate([cos_h, cos_h], axis=0).astype(bf))
    sinT = np.ascontiguousarray(
        np.concatenate([-sin_h, sin_h], axis=0).astype(bf))
    wq = np.asarray(wq, np.float32)
    wk = np.asarray(wk, np.float32)
    wv = np.asarray(wv, np.float32)
    wo = np.asarray(wo, np.float32)
    in_maps = []
    for c in range(NCORES):
        heads = [c + NCORES * r for r in range(NQH)]       # g % KVH == c
        wq_c = np.concatenate(
            [wq[:, g * HD + _ROPE_PERM] for g in heads], axis=1)
        wk_c = wk[:, c * HD + _ROPE_PERM]
        wv_c = wv[:, c * HD:(c + 1) * HD]
        wo_c = np.concatenate([wo[g * HD:(g + 1) * HD, :] for g in heads],
                              axis=0)
        in_maps.append({
            "xT": xTh,
            "wq": np.ascontiguousarray(wq_c.astype(bf)),
            "wk": np.ascontiguousarray(wk_c.astype(bf)),
            "wv": np.ascontiguousarray(wv_c.astype(bf)),
            "wo": np.ascontiguousarray(wo_c.astype(bf)),
            "cosT": cosT,
            "sinT": sinT,
        })
    return in_maps


_NC_CACHE = {}


def _get_nc():
    if "nc" not in _NC_CACHE:
        _NC_CACHE["nc"] = build_attention_kernel()
    return _NC_CACHE["nc"]


def kernel(x, wq, wk, wv, wo, freqs_cos, freqs_sin, mask, cache_k, cache_v,
           start_pos):
    assert int(start_pos) == 0, "kernel assumes prefill at start_pos=0"
    in_maps = shard_inputs(x, wq, wk, wv, wo, freqs_cos, freqs_sin)
    nc = _get_nc()
    res = run_bass_kernel_spmd(nc, in_maps, core_ids=list(range(NCORES)))
    acc = np.zeros((S, D), np.float32)
    for c in range(NCORES):
        acc += res.results[c]["out"]
    return acc.reshape(B, S, D)
